# revision 18
# baseline (speedup 1.0000x reference)
"""AnomalyTransformer forward on 8 trn2 NeuronCores — pure data-parallel over batch.

Feature-major redesign (HW-validated):
  - residual streams hA (attn input) / hB (FFN input) as per-subgroup
    [128, 400] bf16 tiles (WAR deps stay subgroup-local)
  - QKV / Wo / FFN / proj contract with K=128 weight chunks as lhsT; Wo
    consumes head-pair-stacked o2 (AV writes rows 64-127 via
    tile_position=(0,64)); residual adds folded into PE identity-matmuls
  - attention per batch, S^T form: psS[m, l] = K^T Q (+ mask^T via
    identity-lhsT matmul), exp, softmax denominators via ones64 col-tiled
    matmuls, unnormalized AV, normalization fused into the PSUM->SBUF o2
    evacuation (one vector TT against the reciprocal-denominator tile);
    even/odd heads row-tiled T0/T8 into separate PSUM banks (KODD=1)
  - LN feature-major, mean-free-centered: mean broadcast via all-ones/D
    matmul, centered square, sumsq via M=32 ones matmul; rstd magic+Newton
    on [16,25]-packed stats through a DRAM roundtrip; apply = z' *
    rstd-bcast via gpsimd.partition_broadcast
  - act-table discipline: attention pass (exp) and FFN pass (gelu) are
    phase-batched per layer, 2 table switches/layer
"""

import os
import sys

import numpy as np

for _p in ("/opt/trn_rl_repo",):
    if _p not in sys.path:
        sys.path.insert(0, _p)

import ml_dtypes
import concourse.bacc as bacc_mod
import concourse.mybir as mybir
from concourse.tile import TileContext
from concourse.bass_utils import run_bass_kernel_spmd

BF16 = ml_dtypes.bfloat16

B, L, CIN, COUT = 256, 100, 38, 38
D, H, E, DFF = 512, 8, 3, 512
DH = D // H
NC_CORES = 8
BL = B // NC_CORES          # 32 batches per core
T = BL * L                  # 3200 tokens per core
GB = 8                      # batches per attention group
NG = BL // GB               # 4 groups
SGB = 4                     # batches per subgroup (Wo/LN/FFN tile = 400 cols)
NSG = BL // SGB             # 8 subgroups
SGW = SGB * L               # 400
KC = 3 * CIN                # 114 conv contraction rows
NLN = 2 * E + 1             # LN instances

f32 = mybir.dt.float32
f32r = mybir.dt.float32r
fp16 = mybir.dt.float16
bf16 = mybir.dt.bfloat16
i32 = mybir.dt.int32
AF = mybir.ActivationFunctionType
ALU = mybir.AluOpType
AX = mybir.AxisListType

MAGIC_P1 = 0x5F3759DF + 1
KPHASE = int(os.environ.get("KPHASE", "99"))
KFINAL = int(os.environ.get("KFINAL", "1"))
KLN = int(os.environ.get("KLN", "3"))
KATT = int(os.environ.get("KATT", "4"))
KODD = int(os.environ.get("KODD", "1"))   # 1: base-64 operands + row-tile T8


def build_nc(trivial_affine=True, zero_bias=True):
    nc = bacc_mod.Bacc()

    # ---- DRAM parameters ------------------------------------------------
    xaugT = nc.declare_dram_parameter("xaugT", [KC, T], bf16, isOutput=False)
    wcat = nc.declare_dram_parameter("wcat", [KC, D], bf16, isOutput=False)
    petd = nc.declare_dram_parameter("petd", [4, 128, SGW], bf16, isOutput=False)
    wqt = nc.declare_dram_parameter("wqt", [E, 4, 128, D], bf16, isOutput=False)
    wkt = nc.declare_dram_parameter("wkt", [E, 4, 128, D], bf16, isOutput=False)
    wvt = nc.declare_dram_parameter("wvt", [E, 4, 128, D], bf16, isOutput=False)
    wot = nc.declare_dram_parameter("wot", [E, 4, 128, D], bf16, isOutput=False)
    c1wt = nc.declare_dram_parameter("c1wt", [E, 4, 128, D], bf16, isOutput=False)
    c2wt = nc.declare_dram_parameter("c2wt", [E, 4, 128, D], bf16, isOutput=False)
    m01d = nc.declare_dram_parameter("m01d", [L, L], bf16, isOutput=False)
    maskbd = nc.declare_dram_parameter("maskbd", [L, 4 * L], bf16, isOutput=False)
    identd = nc.declare_dram_parameter("identd", [128, 128], bf16, isOutput=False)
    onesdd = nc.declare_dram_parameter("onesdd", [128, 224], bf16, isOutput=False)
    projt = nc.declare_dram_parameter("projt", [4, 128, COUT], bf16, isOutput=False)
    # bias / affine payloads (used only when the fast flags are off)
    biasd = nc.declare_dram_parameter("biasd", [E, 8, D], bf16, isOutput=False)
    projbd = nc.declare_dram_parameter("projbd", [1, COUT], bf16, isOutput=False)
    affd = nc.declare_dram_parameter("affd", [E, 2, 2, 4, 128], f32, isOutput=False)
    out_d = nc.declare_dram_parameter("out", [COUT, T], f32, isOutput=True)

    statsd = nc.declare_dram_parameter("statsd", [NLN, NSG, SGW], f32,
                                       isOutput=True)
    rowd = nc.declare_dram_parameter("rowd", [NLN, T], fp16, isOutput=True)

    with TileContext(nc) as tc:
        with (
            tc.tile_pool(name="const", bufs=1) as cpool,
            tc.tile_pool(name="w", bufs=1) as wpool,
            tc.tile_pool(name="act", bufs=1) as apool,
            tc.tile_pool(name="grp", bufs=2) as gpool,
            tc.tile_pool(name="sc", bufs=3) as spool,
            tc.tile_pool(name="zz", bufs=2) as zpool,
            tc.tile_pool(name="zp", bufs=1) as zppool,
            tc.tile_pool(name="ln", bufs=2) as lpool,
            tc.tile_pool(name="osb", bufs=2) as opool,
            tc.tile_pool(name="ps", bufs=1, space="PSUM") as psum,
        ):
            # ---- embed inputs first (critical path) --------------------
            wcE = cpool.tile([KC, D], bf16, tag="wcE", name="wcE")
            nc.sync.dma_start(out=wcE[:, :], in_=wcat[:, :])
            xaE = cpool.tile([KC, SGW], bf16, tag="xaE", name="xaE")
            nc.sync.dma_start(out=xaE[:, :], in_=xaugT[:, 0:SGW])
            # ---- constants ---------------------------------------------
            idt = cpool.tile([128, 128], bf16, tag="ident", name="ident")
            nc.sync.dma_start(out=idt[:, :], in_=identd[:, :])
            mkb = cpool.tile([L, 4 * L], bf16, tag="mkb", name="mkb")
            nc.sync.dma_start(out=mkb[:, :], in_=maskbd[:, :])
            onesLd = cpool.tile([128, 224], bf16, tag="onesLd", name="onesLd")
            nc.sync.dma_start(out=onesLd[:, :], in_=onesdd[:, :])
            onesDiv = onesLd[:, 0:128]
            ones64 = onesLd[:, 128:192]
            onesP32 = onesLd[:, 192:224]
            if not zero_bias:
                ones1L = cpool.tile([1, L], bf16, tag="ones1L", name="ones1L")
                nc.vector.memset(ones1L[:, :], 1.0)
                onesRow = cpool.tile([1, D], bf16, tag="onesRow",
                                     name="onesRow")
                nc.vector.memset(onesRow[:, :], 1.0)
            pjt = []
            for c in range(4):
                tl = cpool.tile([128, COUT], bf16, tag=f"pjt{c}", name=f"pjt{c}")
                nc.sync.dma_start(out=tl[:, :], in_=projt[c])
                pjt.append(tl)
            pjb = cpool.tile([1, COUT], bf16, tag="pjb", name="pjb")
            nc.sync.dma_start(out=pjb[:, :], in_=projbd[:, :])


            # residual streams, split per subgroup to keep WAR deps local
            hA = [[apool.tile([128, SGW], bf16, tag=f"hA{c}_{s}",
                              name=f"hA{c}_{s}") for s in range(NSG)]
                  for c in range(4)]
            hB = [[apool.tile([128, SGW], bf16, tag=f"hB{c}_{s}",
                              name=f"hB{c}_{s}") for s in range(NSG)]
                  for c in range(4)]

            # round-robin engine pickers
            def tt_eng(i):
                return nc.vector if i % 2 == 0 else nc.gpsimd

            def cp3(i, out, in_):
                if i % 2 == 0:
                    nc.scalar.activation(out, in_, AF.Identity)
                else:
                    nc.vector.tensor_copy(out, in_)

            # ---- LN helper ---------------------------------------------
            def ln_front(ln_id, sg, zsrc_psum, resid, zp_tiles, aff=None):
                """zsrc_psum[c]: PSUM delta tiles [128, SGW] (or None for final);
                resid[c]: residual SBUF [128, T-slice APs]; writes centered z'
                into zp_tiles[c] and sumsq row to statsd[ln_id, sg]."""
                zg = []
                for c in range(4):
                    if zsrc_psum is None:
                        zg.append(resid[c])  # already in SBUF
                    else:
                        t = zpool.tile([128, SGW], bf16, tag=f"zg{c}",
                                       name=f"zg{c}")
                        cp3(c + sg, t[:, :], zsrc_psum[c][:, 0:SGW])
                        zg.append(t[:, :])
                psM = psum.tile([128, 512], f32, tag="ln", name="psM", bufs=1)
                for c in range(4):
                    nc.tensor.matmul(psM[:, 0:SGW], onesDiv[:, :], zg[c],
                                     start=(c == 0), stop=(c == 3))
                mB = zpool.tile([128, SGW], bf16, tag="mB", name="mB")
                cp3(sg, mB[:, :], psM[:, 0:SGW])
                sq = []
                for c in range(4):
                    tt_eng(c + sg).tensor_sub(zp_tiles[c][:, :], zg[c],
                                              mB[:, :])
                    s = zpool.tile([128, SGW], bf16, tag=f"sq{c}", name=f"sq{c}")
                    tt_eng(c + sg + 1).tensor_mul(s[:, :], zp_tiles[c][:, :],
                                                  zp_tiles[c][:, :])
                    sq.append(s)
                psSS = psum.tile([32, 512], f32, tag="ln", name="psSS", bufs=1)
                for c in range(4):
                    nc.tensor.matmul(psSS[:, 0:SGW], onesP32, sq[c][:, :],
                                     start=(c == 0), stop=(c == 3))
                ssb = lpool.tile([1, SGW], f32, tag="ssb", name="ssb", bufs=4)
                cp3(sg, ssb[0:1, 0:SGW], psSS[0:1, 0:SGW])
                nc.sync.dma_start(out=statsd[ln_id, sg].unsqueeze(0),
                                  in_=ssb[0:1, 0:SGW])

            def ln_rstd(ln_id, sg):
                """rstd for one subgroup: packed [16, 25] magic+Newton."""
                ve = nc.vector
                pk = lpool.tile([16, 25], f32, tag="pk", name="pk", bufs=4)
                nc.sync.dma_start(
                    out=pk[:, :],
                    in_=statsd[ln_id, sg].rearrange("(p f) -> p f", p=16))
                w = lpool.tile([16, 25], f32, tag="lnw", name="lnw", bufs=4)
                y = lpool.tile([16, 25], f32, tag="lny", name="lny", bufs=4)
                t1 = lpool.tile([16, 25], f32, tag="lnt", name="lnt", bufs=4)
                ve.tensor_scalar(w[:, :], pk[:, :], 1.0 / D, 1e-5,
                                 op0=ALU.mult, op1=ALU.add)
                wi = w[:, :].bitcast(i32)
                yi = y[:, :].bitcast(i32)
                ti = t1[:, :].bitcast(i32)
                ve.tensor_scalar(ti, wi, 1, None,
                                 op0=ALU.logical_shift_right)
                ve.tensor_scalar(ti, ti, -1, None, op0=ALU.bitwise_xor)
                ve.tensor_scalar(yi, ti, MAGIC_P1, None, op0=ALU.add)
                for _ in range(2):
                    ve.tensor_mul(t1[:, :], y[:, :], y[:, :])
                    ve.tensor_mul(t1[:, :], t1[:, :], w[:, :])
                    ve.tensor_scalar(t1[:, :], t1[:, :], -0.5, 1.5,
                                     op0=ALU.mult, op1=ALU.add)
                    ve.tensor_mul(y[:, :], y[:, :], t1[:, :])
                yh = lpool.tile([16, 25], fp16, tag="lnyh", name="lnyh",
                                bufs=4)
                ve.tensor_copy(yh[:, :], y[:, :])
                nc.sync.dma_start(
                    out=rowd[ln_id, sg * SGW:(sg + 1) * SGW].rearrange(
                        "(p f) -> p f", p=16),
                    in_=yh[:, :])

            def ln_apply(sg, ln_id, zp_tiles, dst, aff=None):
                """dst[c][sg] = zp[c] * rstd-bcast (* gamma + beta)."""
                rw = lpool.tile([1, SGW], fp16, tag="rw", name="rw", bufs=4)
                nc.sync.dma_start(out=rw[0:1, :],
                                  in_=rowd[ln_id, sg * SGW:(sg + 1) * SGW]
                                  .unsqueeze(0))
                rB = lpool.tile([128, SGW], fp16, tag="rB", name="rB", bufs=4)
                nc.gpsimd.partition_broadcast(rB[:, :], rw[0:1, :])
                for c in range(4):
                    dap = dst[c][sg][:, :]
                    tt_eng(c + sg).tensor_mul(dap, zp_tiles[c][:, :], rB[:, :])
                    if aff is not None:
                        nc.vector.tensor_scalar(dap, dap, aff[0][c][:, 0:1],
                                                aff[1][c][:, 0:1],
                                                op0=ALU.mult, op1=ALU.add)

            # ---- embed --------------------------------------------------
            with tc.tile_pool(name="emb", bufs=1) as epool:
                pet = []
                for c in range(4):
                    tl = epool.tile([128, SGW], bf16, tag=f"pet{c}",
                                    name=f"pet{c}")
                    nc.sync.dma_start(out=tl[:, :], in_=petd[c])
                    pet.append(tl)
                wc = wcE
                for sg in range(NSG):
                    cols = slice(sg * SGW, (sg + 1) * SGW)
                    if sg == 0:
                        xa = xaE
                    else:
                        xa = epool.tile([KC, SGW], bf16, tag="xa", name="xa",
                                        bufs=2)
                        nc.sync.dma_start(out=xa[:, :], in_=xaugT[:, cols])
                    for c in range(4):
                        psE = psum.tile([128, 512], f32, tag="mm", name="mm", bufs=3)
                        nc.tensor.matmul(psE[:, 0:SGW],
                                         wc[:, c * 128:(c + 1) * 128],
                                         xa[:, :], start=True, stop=False)
                        nc.tensor.matmul(psE[:, 0:SGW], idt[:, :],
                                         pet[c][:, :], start=False, stop=True)
                        cp3(sg + c, hA[c][sg][:, :], psE[:, 0:SGW])

            # ---- layer weights (per-layer, double-buffered) --------------
            WQ, WK, WV, WO, C1, C2 = {}, {}, {}, {}, {}, {}
            BIAS = {}

            def load_weights(l):
                wop = []
                for p in range(4):
                    tl = wpool.tile([128, D], bf16, tag=f"wop{p}",
                                    name=f"wop{l}{p}")
                    nc.sync.dma_start(out=tl[:, :], in_=wot[l, p])
                    wop.append(tl)
                WO[l] = wop
                for dct, nm, drm in ((WQ, "wq", wqt), (WK, "wk", wkt),
                                     (WV, "wv", wvt),
                                     (C1, "c1", c1wt), (C2, "c2", c2wt)):
                    tiles = []
                    for c in range(4):
                        tl = wpool.tile([128, D], bf16, tag=f"{nm}{c}",
                                        name=f"{nm}{l}{c}")
                        nc.sync.dma_start(out=tl[:, :], in_=drm[l, c])
                        tiles.append(tl)
                    dct[l] = tiles
                if not zero_bias:
                    bt = wpool.tile([8, D], bf16, tag="bias", name=f"bias{l}")
                    nc.sync.dma_start(out=bt[:, :], in_=biasd[l])
                    BIAS[l] = bt
            AFFT = []
            if not trivial_affine:
                for l in range(E):
                    per_ln = []
                    for which in range(2):
                        gs, bs = [], []
                        for c in range(4):
                            g = wpool.tile([128, 1], f32, tag=f"g{l}{which}{c}",
                                           name=f"g{l}{which}{c}")
                            nc.sync.dma_start(out=g[:, :],
                                              in_=affd[l, which, 0, c].unsqueeze(1))
                            bb = wpool.tile([128, 1], f32, tag=f"b{l}{which}{c}",
                                            name=f"b{l}{which}{c}")
                            nc.sync.dma_start(out=bb[:, :],
                                              in_=affd[l, which, 1, c].unsqueeze(1))
                            gs.append(g)
                            bs.append(bb)
                        per_ln.append((gs, bs))
                    AFFT.append(per_ln)

            def bias_row(l, idx):
                # rows: 0 bq,1 bk,2 bv,3 bo,4 c1b,5 c2b
                return BIAS[l][idx:idx + 1, :]

            # ---- per-phase helpers (closures; avoid deep nesting) -------
            def accum_mm(ps, wtiles, rhs_fn, bias_ap):
                for ci in range(4):
                    nc.tensor.matmul(ps, wtiles[ci], rhs_fn(ci),
                                     start=(ci == 0),
                                     stop=(ci == 3 and bias_ap is None))
                if bias_ap is not None:
                    nc.tensor.matmul(ps, bias_ap, onesRow[:, 0:SGW],
                                     start=False, stop=True)

            def do_qkv(l, g):
                Qg = [gpool.tile([128, 2 * SGW], bf16, tag=f"qg{c}",
                                 name=f"qg{c}") for c in range(4)]
                Kg = [gpool.tile([128, 2 * SGW], bf16, tag=f"kg{c}",
                                 name=f"kg{c}") for c in range(4)]
                nqk = 0
                for co in range(4):
                    for hh in range(2):
                        sgv = 2 * g + hh
                        for dst, wt, brow in ((Qg, WQ[l], 0), (Kg, WK[l], 1)):
                            ps = psum.tile([128, 512], f32, tag="mm", name="mm", bufs=3)
                            wts = [wt[ci][:, co * 128:(co + 1) * 128]
                                   for ci in range(4)]
                            bias_ap = (None if zero_bias else
                                       BIAS[l][brow:brow + 1,
                                               co * 128:(co + 1) * 128])
                            accum_mm(ps[:, 0:SGW], wts,
                                     lambda ci: hA[ci][sgv][:, :], bias_ap)
                            cp3(nqk, dst[co][:, hh * SGW:(hh + 1) * SGW],
                                ps[:, 0:SGW])
                            nqk += 1
                if not KODD:
                    Qg2 = [gpool.tile([64, 2 * SGW], bf16, tag=f"qh{c}",
                                      name=f"qh{c}", bufs=1) for c in range(4)]
                    Kg2 = [gpool.tile([64, 2 * SGW], bf16, tag=f"kh{c}",
                                      name=f"kh{c}", bufs=1) for c in range(4)]
                    for c in range(4):
                        nc.sync.dma_start(out=Qg2[c][:, :], in_=Qg[c][64:128, :])
                        nc.sync.dma_start(out=Kg2[c][:, :], in_=Kg[c][64:128, :])
                else:
                    Qg2 = Kg2 = None
                Vg = [gpool.tile([L, D], bf16, tag=f"vg{b}", name=f"vg{b}",
                                 bufs=2) for b in range(GB)]
                for b in range(GB):
                    sgv = 2 * g + b // 4
                    bl = slice((b % 4) * L, (b % 4 + 1) * L)
                    psf = psum.tile([128, 512], f32, tag="mm", name="mm", bufs=3)
                    ps = psf[0:L, :]
                    for ci in range(4):
                        nc.tensor.matmul(ps[:, :], hA[ci][sgv][:, bl],
                                         WV[l][ci],
                                         start=(ci == 0),
                                         stop=(ci == 3 and zero_bias))
                    if not zero_bias:
                        nc.tensor.matmul(ps[:, :], ones1L[:, :],
                                         bias_row(l, 2), start=False, stop=True)
                    cp3(b, Vg[b][:, :], ps[:, :])
                return (Qg, Qg2), (Kg, Kg2), Vg

            def attn_batch(QgT, KgT, Vg, sgh, bj, o2):
                """S^T form: psS*[m, (p, l)] = K^T Q + mask^T per head parity.
                Softmax denominators via ones64 col-tiled matmuls; the
                normalization is fused into the PSUM->SBUF o2 evacuation."""
                Qg, Qg2 = QgT
                Kg, Kg2 = KgT
                b = sgh * SGB + bj
                bc = slice(b * L, (b + 1) * L)
                psSe = psum.tile([L, 512], f32, tag="Se", name="Se", bufs=1)
                psSo = psum.tile([L, 512], f32, tag="So", name="So", bufs=1)
                nc.tensor.matmul(psSe[:, 0:4 * L], idt[0:L, 0:L],
                                 mkb[:, :], start=True, stop=False)
                nc.tensor.matmul(psSo[:, 0:4 * L], idt[0:L, 0:L],
                                 mkb[:, :], start=True, stop=False)
                for co in range(4):
                    cb = co * L
                    nc.tensor.matmul(psSe[:, cb:cb + L], Kg[co][0:64, bc],
                                     Qg[co][0:64, bc], start=False,
                                     stop=(co == 3))
                    if KODD:
                        nc.tensor.matmul(psSo[:, cb:cb + L],
                                         Kg[co][64:128, bc],
                                         Qg[co][64:128, bc], start=False,
                                         stop=(co == 3), tile_position=(64, 0))
                    else:
                        nc.tensor.matmul(psSo[:, cb:cb + L],
                                         Kg2[co][0:64, bc],
                                         Qg2[co][0:64, bc], start=False,
                                         stop=(co == 3))
                e = spool.tile([L, 8 * L], bf16, tag="e", name="e")
                nc.scalar.activation(e[:, 0:4 * L], psSe[:, 0:4 * L], AF.Exp)
                nc.scalar.activation(e[:, 4 * L:8 * L], psSo[:, 0:4 * L],
                                     AF.Exp)
                if KATT < 2:
                    return
                psD = psum.tile([128, 512], f32, tag="D", name="D", bufs=1)
                nc.tensor.matmul(psD[0:64, 0:4 * L], ones64[0:L, :],
                                 e[:, 0:4 * L], start=True, stop=True)
                nc.tensor.matmul(psD[64:128, 0:4 * L], ones64[0:L, :],
                                 e[:, 4 * L:8 * L], start=True, stop=True,
                                 tile_position=(0, 64))
                # r = exp(-ln(d)) on ScalarE: same act-table set as exp
                # (natural_log_exp_and_others); DVE reciprocal is 8 cyc/elem.
                lnD = spool.tile([128, 4 * L], f32, tag="lnD", name="lnD",
                                 bufs=2)
                nc.scalar.activation(lnD[:, :], psD[:, 0:4 * L], AF.Ln)
                rB = spool.tile([128, 4 * L], f32, tag="rB", name="rBatt",
                                bufs=2)
                nc.scalar.activation(rB[:, :], lnD[:, :], AF.Exp, scale=-1.0)
                if KATT < 4:
                    return
                psOb = psum.tile([128, 512], f32, tag="Ob", name="Ob", bufs=1)
                for p in range(4):
                    nc.tensor.matmul(
                        psOb[0:64, p * L:(p + 1) * L],
                        Vg[b][:, (2 * p) * DH:(2 * p + 1) * DH],
                        e[:, p * L:(p + 1) * L],
                        start=True, stop=True)
                    nc.tensor.matmul(
                        psOb[64:128, p * L:(p + 1) * L],
                        Vg[b][:, (2 * p + 1) * DH:(2 * p + 2) * DH],
                        e[:, 4 * L + p * L:4 * L + (p + 1) * L],
                        start=True, stop=True, tile_position=(0, 64))
                nc.vector.tensor_mul(
                    o2[:, :].rearrange("q (p w) -> q p w", p=4)
                    [:, :, bj * L:(bj + 1) * L],
                    psOb[:, 0:4 * L].rearrange("q (p m) -> q p m", p=4),
                    rB[:, :].rearrange("q (p m) -> q p m", p=4))

            def do_wo(l, sg, o2):
                psZ = []
                for co in range(4):
                    ps = psum.tile([128, 512], f32, tag="mm", name="mm", bufs=3)
                    for p in range(4):
                        nc.tensor.matmul(ps[:, 0:SGW],
                                         WO[l][p][:, co * 128:(co + 1) * 128],
                                         o2[:, p * SGW:(p + 1) * SGW],
                                         start=(p == 0), stop=False)
                    if not zero_bias:
                        nc.tensor.matmul(ps[:, 0:SGW],
                                         BIAS[l][3:4, co * 128:(co + 1) * 128],
                                         onesRow[:, 0:SGW], start=False,
                                         stop=False)
                    nc.tensor.matmul(ps[:, 0:SGW], idt[:, :],
                                     hA[co][sg][:, :], start=False, stop=True)
                    psZ.append(ps)
                return psZ

            def do_ffn(l, sg):
                cols = slice(sg * SGW, (sg + 1) * SGW)
                Yg = []
                for co in range(4):
                    # borrow the attention-phase PSUM banks (idle in pass B)
                    ps = psum.tile([128, 512], f32,
                                   tag=("Se", "So", "D", "Ob")[co],
                                   name="ffn1", bufs=1)
                    wts = [C1[l][ci][:, co * 128:(co + 1) * 128]
                           for ci in range(4)]
                    bias_ap = (None if zero_bias else
                               BIAS[l][4:5, co * 128:(co + 1) * 128])
                    accum_mm(ps[:, 0:SGW], wts,
                             lambda ci: hB[ci][sg][:, :], bias_ap)
                    yt = zpool.tile([128, SGW], bf16, tag=f"y{co}",
                                    name=f"y{co}")
                    nc.scalar.activation(yt[:, :], ps[:, 0:SGW], AF.Gelu)
                    Yg.append(yt)
                psZ2 = []
                for co in range(4):
                    ps = psum.tile([128, 512], f32, tag="mm", name="mm", bufs=3)
                    for ci in range(4):
                        nc.tensor.matmul(ps[:, 0:SGW],
                                         C2[l][ci][:, co * 128:(co + 1) * 128],
                                         Yg[ci][:, :], start=(ci == 0),
                                         stop=False)
                    if not zero_bias:
                        nc.tensor.matmul(ps[:, 0:SGW],
                                         BIAS[l][5:6, co * 128:(co + 1) * 128],
                                         onesRow[:, 0:SGW], start=False,
                                         stop=False)
                    nc.tensor.matmul(ps[:, 0:SGW], idt[:, :],
                                     hB[co][sg][:, :], start=False, stop=True)
                    psZ2.append(ps)
                return psZ2

            # ---- layers (subgroup-local pipeline) -----------------------
            for l in range(E if KPHASE >= 2 else 0):
                ln1, ln2 = 2 * l, 2 * l + 1
                load_weights(l)
                # pass A: attention + LN1 for all subgroups (exp act-table)
                for g in range(NG):
                    Qg, Kg, Vg = do_qkv(l, g)
                    if KPHASE < 3:
                        continue
                    for sgh in range(2):
                        sg = 2 * g + sgh
                        o2 = gpool.tile([128, 4 * SGW], bf16, tag=f"o2{sgh}",
                                        name=f"o2{sgh}", bufs=2)
                        for bj in range(SGB):
                            attn_batch(Qg, Kg, Vg, sgh, bj, o2)
                        if KPHASE < 4:
                            continue
                        psZ = do_wo(l, sg, o2)
                        if KPHASE < 5:
                            continue
                        zp = [zppool.tile([128, SGW], bf16,
                                          tag=f"zpA_{sg % 4}_{c}",
                                          name=f"zpA{l}_{sg}_{c}")
                              for c in range(4)]
                        ln_front(ln1, sg, psZ,
                                 [hA[c][sg][:, :] for c in range(4)], zp)
                        if KLN >= 2:
                            ln_rstd(ln1, sg)
                        if KLN >= 3:
                            ln_apply(sg, ln1, zp, hB,
                                     None if trivial_affine else AFFT[l][0])
                if KPHASE < 6:
                    continue
                # pass B: FFN + LN2 for all subgroups (gelu act-table)
                for sg in range(NSG):
                    psZ2 = do_ffn(l, sg)
                    zp2 = [zppool.tile([128, SGW], bf16,
                                       tag=f"zpA_{sg % 4}_{c}",
                                       name=f"zpB{l}_{sg}_{c}")
                           for c in range(4)]
                    ln_front(ln2, sg, psZ2,
                             [hB[c][sg][:, :] for c in range(4)], zp2)
                    ln_rstd(ln2, sg)
                    ln_apply(sg, ln2, zp2, hA,
                             None if trivial_affine else AFFT[l][1])
            # end layers

            # ---- final LN + projection ---------------------------------
            if KPHASE >= 2 and KFINAL:
                lnf = 2 * E
                for sg in range(NSG):
                    cols = slice(sg * SGW, (sg + 1) * SGW)
                    zpf = [zppool.tile([128, SGW], bf16,
                                       tag=f"zpF_{sg % 2}_{c}",
                                       name=f"zpf_{sg}_{c}")
                           for c in range(4)]
                    ln_front(lnf, sg, None,
                             [hA[c][sg][:, :] for c in range(4)], zpf)
                    ln_rstd(lnf, sg)
                    rw = lpool.tile([1, SGW], fp16, tag="rw", name="rw",
                                    bufs=4)
                    nc.sync.dma_start(
                        out=rw[0:1, :],
                        in_=rowd[lnf, sg * SGW:(sg + 1) * SGW].unsqueeze(0))
                    rB = lpool.tile([128, SGW], fp16, tag="rB", name="rB",
                                    bufs=4)
                    nc.gpsimd.partition_broadcast(rB[:, :], rw[0:1, :])
                    psf = psum.tile([128, 512], f32,
                                    tag=("Se", "So", "D", "Ob")[sg % 4],
                                    name="proj", bufs=1)
                    ps = psf[0:COUT, :]
                    for ci in range(4):
                        nc.tensor.matmul(ps[:, 0:SGW], pjt[ci][:, :],
                                         zpf[ci][:, :],
                                         start=(ci == 0),
                                         stop=(ci == 3 and zero_bias))
                    if not zero_bias:
                        nc.tensor.matmul(ps[:, 0:SGW], pjb[:, :],
                                         onesRow[:, 0:SGW],
                                         start=False, stop=True)
                    osb = opool.tile([COUT, SGW], f32, tag="osb", name="osb")
                    nc.vector.tensor_mul(osb[:, :], ps[0:COUT, 0:SGW],
                                         rB[0:COUT, :])
                    nc.sync.dma_start(out=out_d[:, cols], in_=osb[:, :])
            if KPHASE < 2 or not KFINAL:
                # debug: dump embed output
                for sg in range(NSG):
                    cols = slice(sg * SGW, (sg + 1) * SGW)
                    osb = opool.tile([COUT, SGW], f32, tag="osb", name="osb")
                    nc.vector.tensor_copy(osb[:, :], hA[0][sg][0:COUT, :])
                    nc.sync.dma_start(out=out_d[:, cols], in_=osb[:, :])

    nc.compile()
    return nc


# ---------------------------------------------------------------------------
# host side
# ---------------------------------------------------------------------------

def _pos_encoding():
    pos = np.arange(L)[:, None].astype(np.float32)
    div = np.exp(np.arange(0, D, 2).astype(np.float32) * (-np.log(10000.0) / D))
    pe = np.zeros((L, D), dtype=np.float32)
    pe[:, 0::2] = np.sin(pos * div)
    pe[:, 1::2] = np.cos(pos * div)
    return pe


def _chunk4(mT):
    """[D, N] -> [4, 128, N]"""
    return np.ascontiguousarray(mT.reshape(4, 128, -1))


_NC = None
_NC_FLAGS = None


def _get_nc(trivial_affine=True, zero_bias=True):
    global _NC, _NC_FLAGS
    if _NC is None or _NC_FLAGS != (trivial_affine, zero_bias):
        _NC = build_nc(trivial_affine, zero_bias)
        _NC_FLAGS = (trivial_affine, zero_bias)
    return _NC


def is_trivial_affine(inputs):
    i = {k: np.asarray(v) for k, v in inputs.items()}
    return (np.all(i["ln1s"] == 1.0) and np.all(i["ln1b"] == 0.0)
            and np.all(i["ln2s"] == 1.0) and np.all(i["ln2b"] == 0.0))


def is_zero_bias(inputs):
    i = {k: np.asarray(v) for k, v in inputs.items()}
    return all(bool(np.all(i[k] == 0.0))
               for k in ("bq", "bk", "bv", "bo", "c1b", "c2b", "proj_b",
                         "lnfb"))


def prepare_maps(inputs):
    inp = {k: np.asarray(v) for k, v in inputs.items()}
    x = inp["x"].astype(np.float32)
    emb_w = inp["emb_w"].astype(np.float32)
    mask = inp["mask"].astype(np.float32)

    scale = 1.0 / np.sqrt(DH)

    wqt = np.stack([_chunk4(inp["Wq"][l].T * scale) for l in range(E)]).astype(BF16)
    wkt = np.stack([_chunk4(inp["Wk"][l].T) for l in range(E)]).astype(BF16)
    wvt = np.stack([_chunk4(inp["Wv"][l].T) for l in range(E)]).astype(BF16)
    wot = np.stack([_chunk4(inp["Wo"][l].T) for l in range(E)]).astype(BF16)
    c1wt = np.stack([_chunk4(inp["c1w"][l].T) for l in range(E)]).astype(BF16)
    c2wt = np.stack([_chunk4(inp["c2w"][l].T) for l in range(E)]).astype(BF16)

    biasd = np.zeros((E, 8, D), np.float32)
    for l in range(E):
        biasd[l, 0] = inp["bq"][l] * scale
        biasd[l, 1] = inp["bk"][l]
        biasd[l, 2] = inp["bv"][l]
        biasd[l, 3] = inp["bo"][l]
        biasd[l, 4] = inp["c1b"][l]
        biasd[l, 5] = inp["c2b"][l]
    affd = np.zeros((E, 2, 2, 4, 128), np.float32)
    for l in range(E):
        affd[l, 0, 0] = inp["ln1s"][l].reshape(4, 128)
        affd[l, 0, 1] = inp["ln1b"][l].reshape(4, 128)
        affd[l, 1, 0] = inp["ln2s"][l].reshape(4, 128)
        affd[l, 1, 1] = inp["ln2b"][l].reshape(4, 128)

    projw_eff = inp["proj_w"] * inp["lnfs"][None, :]
    projb_eff = inp["proj_b"] + inp["lnfb"] @ inp["proj_w"].T
    projt = np.ascontiguousarray(projw_eff.T.reshape(4, 128, COUT)).astype(BF16)

    pet = np.ascontiguousarray(
        np.tile(_pos_encoding().T.reshape(4, 128, L), (1, 1, SGB))).astype(BF16)
    wcat = np.concatenate([emb_w[:, :, 0].T, emb_w[:, :, 1].T,
                           emb_w[:, :, 2].T], axis=0)
    ident = np.eye(128, dtype=np.float32).astype(BF16)

    onesdd = np.concatenate([np.full((128, 128), 1.0 / D, np.float32),
                             np.ones((128, 64), np.float32),
                             np.ones((128, 32), np.float32)], axis=1)
    shared = dict(
        onesdd=onesdd.astype(BF16),
        wcat=wcat.astype(BF16), petd=pet, wqt=wqt, wkt=wkt, wvt=wvt, wot=wot,
        c1wt=c1wt, c2wt=c2wt, m01d=mask.astype(BF16),
        maskbd=np.tile(-30.0 * (1.0 - mask).T, (1, 4)).astype(BF16),
        identd=ident,
        projt=projt, biasd=biasd.astype(BF16),
        projbd=projb_eff.reshape(1, COUT).astype(BF16), affd=affd,
    )

    in_maps = []
    for ci in range(NC_CORES):
        xs = x[ci * BL:(ci + 1) * BL]                      # [32, 100, 38]
        xp = np.concatenate([xs[:, -1:], xs, xs[:, :1]], axis=1)  # [32,102,38]
        feats = [xp[:, w:w + L, :] for w in range(3)]      # each [32,100,38]
        xaug = np.concatenate(feats, axis=2)               # [32,100,114]
        xaugT = np.ascontiguousarray(
            xaug.reshape(T, KC).T).astype(BF16)            # [114, 3200]
        m = dict(shared)
        m["xaugT"] = xaugT
        in_maps.append(m)
    return in_maps


def run(inputs, **kw):
    nc = _get_nc(is_trivial_affine(inputs), is_zero_bias(inputs))
    in_maps = prepare_maps(inputs)
    res = run_bass_kernel_spmd(nc, in_maps, core_ids=list(range(NC_CORES)), **kw)
    outs = []
    for ci in range(NC_CORES):
        o = np.asarray(res.results[ci]["out"], np.float32)  # [38, 3200]
        outs.append(o.T.reshape(BL, L, COUT))
    full = np.concatenate(outs, axis=0)
    return full, res


def kernel(**inputs):
    full, _ = run(inputs)
    return full.astype(np.float32)


def bench(inputs, iters=6):
    """Steady-state wall timing of the sharded jitted executable."""
    import time
    import jax
    from jax.sharding import Mesh, PartitionSpec
    from jax.experimental.shard_map import shard_map
    from concourse import mybir
    from concourse.bass2jax import _bass_exec_p, install_neuronx_cc_hook, partition_id_tensor

    nc = _get_nc(is_trivial_affine(inputs), is_zero_bias(inputs))
    in_maps = prepare_maps(inputs)
    install_neuronx_cc_hook()
    partition_name = nc.partition_id_tensor.name if nc.partition_id_tensor else None
    in_names, out_names, out_avals, zero_outs = [], [], [], []
    for alloc in nc.m.functions[0].allocations:
        if not isinstance(alloc, mybir.MemoryLocationSet):
            continue
        name = alloc.memorylocations[0].name
        if alloc.kind == "ExternalInput":
            if name != partition_name:
                in_names.append(name)
        elif alloc.kind == "ExternalOutput":
            out_names.append(name)
            shape = tuple(alloc.tensor_shape)
            dtype = mybir.dt.np(alloc.dtype)
            out_avals.append(jax.core.ShapedArray(shape, dtype))
            zero_outs.append(np.zeros(shape, dtype))
    n_params = len(in_names)
    n_outs = len(out_avals)
    all_names = list(in_names) + out_names + ([partition_name] if partition_name else [])

    def _body(*args):
        operands = list(args)
        if partition_name is not None:
            operands.append(partition_id_tensor())
        return tuple(_bass_exec_p.bind(
            *operands, out_avals=tuple(out_avals), in_names=tuple(all_names),
            out_names=tuple(out_names), lowering_input_output_aliases=(),
            sim_require_finite=True, sim_require_nnan=True, nc=nc))

    devices = jax.devices()[:NC_CORES]
    mesh = Mesh(np.array(devices), ("core",))
    donate = tuple(range(n_params, n_params + n_outs))
    sharded = jax.jit(
        shard_map(_body, mesh=mesh,
                  in_specs=(PartitionSpec("core"),) * (n_params + n_outs),
                  out_specs=(PartitionSpec("core"),) * n_outs,
                  check_rep=False),
        donate_argnums=donate, keep_unused=True)
    concat_in = [np.concatenate([np.asarray(in_maps[c][n]) for c in range(NC_CORES)], axis=0)
                 for n in in_names]
    dev_in = [jax.device_put(a) for a in concat_in]
    times = []
    out = None
    for it in range(iters):
        zeros = [jax.device_put(np.zeros((NC_CORES * z.shape[0], *z.shape[1:]), z.dtype))
                 for z in zero_outs]
        jax.block_until_ready(zeros)
        t0 = time.perf_counter()
        out = sharded(*dev_in, *zeros)
        jax.block_until_ready(out)
        times.append(time.perf_counter() - t0)
    res = np.asarray(out[0]).reshape(NC_CORES, COUT, T)
    full = np.concatenate([res[c].T.reshape(BL, L, COUT) for c in range(NC_CORES)], axis=0)
    return full, times



# revision 19
# speedup vs baseline: 1.1157x; 1.1157x over previous
"""AnomalyTransformer forward on 8 trn2 NeuronCores — pure data-parallel over batch.

Feature-major redesign (HW-validated):
  - residual streams hA (attn input) / hB (FFN input) as per-subgroup
    [128, 400] bf16 tiles (WAR deps stay subgroup-local)
  - QKV / Wo / FFN / proj contract with K=128 weight chunks as lhsT; Wo
    consumes head-pair-stacked o2 (AV writes rows 64-127 via
    tile_position=(0,64)); residual adds folded into PE identity-matmuls
  - attention per batch, S^T form: psS[m, l] = K^T Q (+ mask^T via
    identity-lhsT matmul), exp, softmax denominators via ones64 col-tiled
    matmuls, unnormalized AV, normalization fused into the PSUM->SBUF o2
    evacuation (one vector TT against the reciprocal-denominator tile);
    even/odd heads row-tiled T0/T8 into separate PSUM banks (KODD=1)
  - LN feature-major, mean-free-centered: mean broadcast via all-ones/D
    matmul, centered square, sumsq via M=32 ones matmul; rstd magic+Newton
    on [16,25]-packed stats through a DRAM roundtrip; apply = z' *
    rstd-bcast via gpsimd.partition_broadcast
  - act-table discipline: attention pass (exp) and FFN pass (gelu) are
    phase-batched per layer, 2 table switches/layer
"""

import os
import sys

import numpy as np

for _p in ("/opt/trn_rl_repo",):
    if _p not in sys.path:
        sys.path.insert(0, _p)

import ml_dtypes
import concourse.bacc as bacc_mod
import concourse.mybir as mybir
from concourse.tile import TileContext
from concourse.bass_utils import run_bass_kernel_spmd

# Act-table steering: the table chooser binds Exp -> exp_and_others and
# Ln -> natural_log (first set containing each fn), which thrashes
# ACT_TABLE_LOADs when the softmax uses r = exp(-ln(denom)). Empty those
# sets (keys stay, so act_func_set_ids stay aligned with act_info.json)
# so both Exp and Ln resolve to natural_log_exp_and_others.
import concourse.hw_specs as _hw_specs


def _patched_gat(arch, _orig=_hw_specs.get_activation_tables):
    t = _orig(arch)
    for k in ("exp_and_others", "natural_log", "exp_and_friends"):
        if k in t:
            t[k] = set()
    return t


_hw_specs.get_activation_tables = _patched_gat
if getattr(bacc_mod, "get_activation_tables", None) is not None:
    bacc_mod.get_activation_tables = _patched_gat

BF16 = ml_dtypes.bfloat16

B, L, CIN, COUT = 256, 100, 38, 38
D, H, E, DFF = 512, 8, 3, 512
DH = D // H
NC_CORES = 8
BL = B // NC_CORES          # 32 batches per core
T = BL * L                  # 3200 tokens per core
GB = 8                      # batches per attention group
NG = BL // GB               # 4 groups
SGB = 4                     # batches per subgroup (Wo/LN/FFN tile = 400 cols)
NSG = BL // SGB             # 8 subgroups
SGW = SGB * L               # 400
KC = 3 * CIN                # 114 conv contraction rows
NLN = 2 * E + 1             # LN instances

f32 = mybir.dt.float32
f32r = mybir.dt.float32r
fp16 = mybir.dt.float16
bf16 = mybir.dt.bfloat16
i32 = mybir.dt.int32
AF = mybir.ActivationFunctionType
ALU = mybir.AluOpType
AX = mybir.AxisListType

MAGIC_P1 = 0x5F3759DF + 1
KPHASE = int(os.environ.get("KPHASE", "99"))
KFINAL = int(os.environ.get("KFINAL", "1"))
KLN = int(os.environ.get("KLN", "3"))
KATT = int(os.environ.get("KATT", "4"))
KODD = int(os.environ.get("KODD", "1"))   # 1: base-64 operands + row-tile T8


def build_nc(trivial_affine=True, zero_bias=True):
    nc = bacc_mod.Bacc()

    # ---- DRAM parameters ------------------------------------------------
    xaugT = nc.declare_dram_parameter("xaugT", [KC, T], bf16, isOutput=False)
    wcat = nc.declare_dram_parameter("wcat", [KC, D], bf16, isOutput=False)
    petd = nc.declare_dram_parameter("petd", [4, 128, SGW], bf16, isOutput=False)
    wqt = nc.declare_dram_parameter("wqt", [E, 4, 128, D], bf16, isOutput=False)
    wkt = nc.declare_dram_parameter("wkt", [E, 4, 128, D], bf16, isOutput=False)
    wvt = nc.declare_dram_parameter("wvt", [E, 4, 128, D], bf16, isOutput=False)
    wot = nc.declare_dram_parameter("wot", [E, 4, 128, D], bf16, isOutput=False)
    c1wt = nc.declare_dram_parameter("c1wt", [E, 4, 128, D], bf16, isOutput=False)
    c2wt = nc.declare_dram_parameter("c2wt", [E, 4, 128, D], bf16, isOutput=False)
    m01d = nc.declare_dram_parameter("m01d", [L, L], bf16, isOutput=False)
    maskbd = nc.declare_dram_parameter("maskbd", [L, 4 * L], bf16, isOutput=False)
    identd = nc.declare_dram_parameter("identd", [128, 128], bf16, isOutput=False)
    onesdd = nc.declare_dram_parameter("onesdd", [128, 224], bf16, isOutput=False)
    projt = nc.declare_dram_parameter("projt", [4, 128, COUT], bf16, isOutput=False)
    # bias / affine payloads (used only when the fast flags are off)
    biasd = nc.declare_dram_parameter("biasd", [E, 8, D], bf16, isOutput=False)
    projbd = nc.declare_dram_parameter("projbd", [1, COUT], bf16, isOutput=False)
    affd = nc.declare_dram_parameter("affd", [E, 2, 2, 4, 128], f32, isOutput=False)
    out_d = nc.declare_dram_parameter("out", [COUT, T], f32, isOutput=True)

    statsd = nc.declare_dram_parameter("statsd", [NLN, NSG, SGW], f32,
                                       isOutput=True)
    rowd = nc.declare_dram_parameter("rowd", [NLN, T], fp16, isOutput=True)

    with TileContext(nc) as tc:
        with (
            tc.tile_pool(name="const", bufs=1) as cpool,
            tc.tile_pool(name="w", bufs=1) as wpool,
            tc.tile_pool(name="act", bufs=1) as apool,
            tc.tile_pool(name="grp", bufs=2) as gpool,
            tc.tile_pool(name="sc", bufs=3) as spool,
            tc.tile_pool(name="zz", bufs=2) as zpool,
            tc.tile_pool(name="zp", bufs=1) as zppool,
            tc.tile_pool(name="ln", bufs=2) as lpool,
            tc.tile_pool(name="osb", bufs=2) as opool,
            tc.tile_pool(name="ps", bufs=1, space="PSUM") as psum,
        ):
            # ---- embed inputs first (critical path) --------------------
            wcE = cpool.tile([KC, D], bf16, tag="wcE", name="wcE")
            nc.sync.dma_start(out=wcE[:, :], in_=wcat[:, :])
            xaE = cpool.tile([KC, SGW], bf16, tag="xaE", name="xaE")
            nc.sync.dma_start(out=xaE[:, :], in_=xaugT[:, 0:SGW])
            # ---- constants ---------------------------------------------
            idt = cpool.tile([128, 128], bf16, tag="ident", name="ident")
            nc.sync.dma_start(out=idt[:, :], in_=identd[:, :])
            mkb = cpool.tile([L, 4 * L], bf16, tag="mkb", name="mkb")
            nc.sync.dma_start(out=mkb[:, :], in_=maskbd[:, :])
            onesLd = cpool.tile([128, 224], bf16, tag="onesLd", name="onesLd")
            nc.sync.dma_start(out=onesLd[:, :], in_=onesdd[:, :])
            onesDiv = onesLd[:, 0:128]
            ones64 = onesLd[:, 128:192]
            onesP32 = onesLd[:, 192:224]
            if not zero_bias:
                ones1L = cpool.tile([1, L], bf16, tag="ones1L", name="ones1L")
                nc.vector.memset(ones1L[:, :], 1.0)
                onesRow = cpool.tile([1, D], bf16, tag="onesRow",
                                     name="onesRow")
                nc.vector.memset(onesRow[:, :], 1.0)
            pjt = []
            for c in range(4):
                tl = cpool.tile([128, COUT], bf16, tag=f"pjt{c}", name=f"pjt{c}")
                nc.sync.dma_start(out=tl[:, :], in_=projt[c])
                pjt.append(tl)
            pjb = cpool.tile([1, COUT], bf16, tag="pjb", name="pjb")
            nc.sync.dma_start(out=pjb[:, :], in_=projbd[:, :])


            # residual streams, split per subgroup to keep WAR deps local
            hA = [[apool.tile([128, SGW], bf16, tag=f"hA{c}_{s}",
                              name=f"hA{c}_{s}") for s in range(NSG)]
                  for c in range(4)]
            hB = [[apool.tile([128, SGW], bf16, tag=f"hB{c}_{s}",
                              name=f"hB{c}_{s}") for s in range(NSG)]
                  for c in range(4)]

            # round-robin engine pickers
            def tt_eng(i):
                return nc.vector if i % 2 == 0 else nc.gpsimd

            def cp3(i, out, in_):
                if i % 2 == 0:
                    nc.scalar.activation(out, in_, AF.Identity)
                else:
                    nc.vector.tensor_copy(out, in_)

            # ---- LN helper ---------------------------------------------
            def ln_front(ln_id, sg, zsrc_psum, resid, zp_tiles, aff=None):
                """zsrc_psum[c]: PSUM delta tiles [128, SGW] (or None for final);
                resid[c]: residual SBUF [128, T-slice APs]; writes centered z'
                into zp_tiles[c] and sumsq row to statsd[ln_id, sg]."""
                zg = []
                for c in range(4):
                    if zsrc_psum is None:
                        zg.append(resid[c])  # already in SBUF
                    else:
                        t = zpool.tile([128, SGW], bf16, tag=f"zg{c}",
                                       name=f"zg{c}")
                        cp3(c + sg, t[:, :], zsrc_psum[c][:, 0:SGW])
                        zg.append(t[:, :])
                psM = psum.tile([128, 512], f32, tag="ln", name="psM", bufs=1)
                for c in range(4):
                    nc.tensor.matmul(psM[:, 0:SGW], onesDiv[:, :], zg[c],
                                     start=(c == 0), stop=(c == 3))
                mB = zpool.tile([128, SGW], bf16, tag="mB", name="mB")
                cp3(sg, mB[:, :], psM[:, 0:SGW])
                sq = []
                for c in range(4):
                    tt_eng(c + sg).tensor_sub(zp_tiles[c][:, :], zg[c],
                                              mB[:, :])
                    s = zpool.tile([128, SGW], bf16, tag=f"sq{c}", name=f"sq{c}")
                    tt_eng(c + sg + 1).tensor_mul(s[:, :], zp_tiles[c][:, :],
                                                  zp_tiles[c][:, :])
                    sq.append(s)
                psSS = psum.tile([32, 512], f32, tag="ln", name="psSS", bufs=1)
                for c in range(4):
                    nc.tensor.matmul(psSS[:, 0:SGW], onesP32, sq[c][:, :],
                                     start=(c == 0), stop=(c == 3))
                ssb = lpool.tile([1, SGW], f32, tag="ssb", name="ssb", bufs=4)
                cp3(sg, ssb[0:1, 0:SGW], psSS[0:1, 0:SGW])
                nc.sync.dma_start(out=statsd[ln_id, sg].unsqueeze(0),
                                  in_=ssb[0:1, 0:SGW])

            def ln_rstd(ln_id, sg):
                """rstd for one subgroup: packed [16, 25] magic+Newton."""
                ve = nc.vector
                pk = lpool.tile([16, 25], f32, tag="pk", name="pk", bufs=4)
                nc.sync.dma_start(
                    out=pk[:, :],
                    in_=statsd[ln_id, sg].rearrange("(p f) -> p f", p=16))
                w = lpool.tile([16, 25], f32, tag="lnw", name="lnw", bufs=4)
                y = lpool.tile([16, 25], f32, tag="lny", name="lny", bufs=4)
                t1 = lpool.tile([16, 25], f32, tag="lnt", name="lnt", bufs=4)
                ve.tensor_scalar(w[:, :], pk[:, :], 1.0 / D, 1e-5,
                                 op0=ALU.mult, op1=ALU.add)
                wi = w[:, :].bitcast(i32)
                yi = y[:, :].bitcast(i32)
                ti = t1[:, :].bitcast(i32)
                ve.tensor_scalar(ti, wi, 1, None,
                                 op0=ALU.logical_shift_right)
                ve.tensor_scalar(ti, ti, -1, None, op0=ALU.bitwise_xor)
                ve.tensor_scalar(yi, ti, MAGIC_P1, None, op0=ALU.add)
                for _ in range(2):
                    ve.tensor_mul(t1[:, :], y[:, :], y[:, :])
                    ve.tensor_mul(t1[:, :], t1[:, :], w[:, :])
                    ve.tensor_scalar(t1[:, :], t1[:, :], -0.5, 1.5,
                                     op0=ALU.mult, op1=ALU.add)
                    ve.tensor_mul(y[:, :], y[:, :], t1[:, :])
                yh = lpool.tile([16, 25], fp16, tag="lnyh", name="lnyh",
                                bufs=4)
                ve.tensor_copy(yh[:, :], y[:, :])
                nc.sync.dma_start(
                    out=rowd[ln_id, sg * SGW:(sg + 1) * SGW].rearrange(
                        "(p f) -> p f", p=16),
                    in_=yh[:, :])

            def ln_apply(sg, ln_id, zp_tiles, dst, aff=None):
                """dst[c][sg] = zp[c] * rstd-bcast (* gamma + beta)."""
                rw = lpool.tile([1, SGW], fp16, tag="rw", name="rw", bufs=4)
                nc.sync.dma_start(out=rw[0:1, :],
                                  in_=rowd[ln_id, sg * SGW:(sg + 1) * SGW]
                                  .unsqueeze(0))
                rB = lpool.tile([128, SGW], fp16, tag="rB", name="rB", bufs=4)
                nc.gpsimd.partition_broadcast(rB[:, :], rw[0:1, :])
                for c in range(4):
                    dap = dst[c][sg][:, :]
                    tt_eng(c + sg).tensor_mul(dap, zp_tiles[c][:, :], rB[:, :])
                    if aff is not None:
                        nc.vector.tensor_scalar(dap, dap, aff[0][c][:, 0:1],
                                                aff[1][c][:, 0:1],
                                                op0=ALU.mult, op1=ALU.add)

            # ---- embed --------------------------------------------------
            with tc.tile_pool(name="emb", bufs=1) as epool:
                pet = []
                for c in range(4):
                    tl = epool.tile([128, SGW], bf16, tag=f"pet{c}",
                                    name=f"pet{c}")
                    nc.sync.dma_start(out=tl[:, :], in_=petd[c])
                    pet.append(tl)
                wc = wcE
                for sg in range(NSG):
                    cols = slice(sg * SGW, (sg + 1) * SGW)
                    if sg == 0:
                        xa = xaE
                    else:
                        xa = epool.tile([KC, SGW], bf16, tag="xa", name="xa",
                                        bufs=2)
                        nc.sync.dma_start(out=xa[:, :], in_=xaugT[:, cols])
                    for c in range(4):
                        psE = psum.tile([128, 512], f32, tag="mm", name="mm", bufs=3)
                        nc.tensor.matmul(psE[:, 0:SGW],
                                         wc[:, c * 128:(c + 1) * 128],
                                         xa[:, :], start=True, stop=False)
                        nc.tensor.matmul(psE[:, 0:SGW], idt[:, :],
                                         pet[c][:, :], start=False, stop=True)
                        cp3(sg + c, hA[c][sg][:, :], psE[:, 0:SGW])

            # ---- layer weights (per-layer, double-buffered) --------------
            WQ, WK, WV, WO, C1, C2 = {}, {}, {}, {}, {}, {}
            BIAS = {}

            def load_weights(l):
                wop = []
                for p in range(4):
                    tl = wpool.tile([128, D], bf16, tag=f"wop{p}",
                                    name=f"wop{l}{p}")
                    nc.sync.dma_start(out=tl[:, :], in_=wot[l, p])
                    wop.append(tl)
                WO[l] = wop
                for dct, nm, drm in ((WQ, "wq", wqt), (WK, "wk", wkt),
                                     (WV, "wv", wvt),
                                     (C1, "c1", c1wt), (C2, "c2", c2wt)):
                    tiles = []
                    for c in range(4):
                        tl = wpool.tile([128, D], bf16, tag=f"{nm}{c}",
                                        name=f"{nm}{l}{c}")
                        nc.sync.dma_start(out=tl[:, :], in_=drm[l, c])
                        tiles.append(tl)
                    dct[l] = tiles
                if not zero_bias:
                    bt = wpool.tile([8, D], bf16, tag="bias", name=f"bias{l}")
                    nc.sync.dma_start(out=bt[:, :], in_=biasd[l])
                    BIAS[l] = bt
            AFFT = []
            if not trivial_affine:
                for l in range(E):
                    per_ln = []
                    for which in range(2):
                        gs, bs = [], []
                        for c in range(4):
                            g = wpool.tile([128, 1], f32, tag=f"g{l}{which}{c}",
                                           name=f"g{l}{which}{c}")
                            nc.sync.dma_start(out=g[:, :],
                                              in_=affd[l, which, 0, c].unsqueeze(1))
                            bb = wpool.tile([128, 1], f32, tag=f"b{l}{which}{c}",
                                            name=f"b{l}{which}{c}")
                            nc.sync.dma_start(out=bb[:, :],
                                              in_=affd[l, which, 1, c].unsqueeze(1))
                            gs.append(g)
                            bs.append(bb)
                        per_ln.append((gs, bs))
                    AFFT.append(per_ln)

            def bias_row(l, idx):
                # rows: 0 bq,1 bk,2 bv,3 bo,4 c1b,5 c2b
                return BIAS[l][idx:idx + 1, :]

            # ---- per-phase helpers (closures; avoid deep nesting) -------
            def accum_mm(ps, wtiles, rhs_fn, bias_ap):
                for ci in range(4):
                    nc.tensor.matmul(ps, wtiles[ci], rhs_fn(ci),
                                     start=(ci == 0),
                                     stop=(ci == 3 and bias_ap is None))
                if bias_ap is not None:
                    nc.tensor.matmul(ps, bias_ap, onesRow[:, 0:SGW],
                                     start=False, stop=True)

            def do_qkv(l, g):
                Qg = [gpool.tile([128, 2 * SGW], bf16, tag=f"qg{c}",
                                 name=f"qg{c}") for c in range(4)]
                Kg = [gpool.tile([128, 2 * SGW], bf16, tag=f"kg{c}",
                                 name=f"kg{c}") for c in range(4)]
                nqk = 0
                for co in range(4):
                    for hh in range(2):
                        sgv = 2 * g + hh
                        for dst, wt, brow in ((Qg, WQ[l], 0), (Kg, WK[l], 1)):
                            ps = psum.tile([128, 512], f32, tag="mm", name="mm", bufs=3)
                            wts = [wt[ci][:, co * 128:(co + 1) * 128]
                                   for ci in range(4)]
                            bias_ap = (None if zero_bias else
                                       BIAS[l][brow:brow + 1,
                                               co * 128:(co + 1) * 128])
                            accum_mm(ps[:, 0:SGW], wts,
                                     lambda ci: hA[ci][sgv][:, :], bias_ap)
                            cp3(nqk, dst[co][:, hh * SGW:(hh + 1) * SGW],
                                ps[:, 0:SGW])
                            nqk += 1
                if not KODD:
                    Qg2 = [gpool.tile([64, 2 * SGW], bf16, tag=f"qh{c}",
                                      name=f"qh{c}", bufs=1) for c in range(4)]
                    Kg2 = [gpool.tile([64, 2 * SGW], bf16, tag=f"kh{c}",
                                      name=f"kh{c}", bufs=1) for c in range(4)]
                    for c in range(4):
                        nc.sync.dma_start(out=Qg2[c][:, :], in_=Qg[c][64:128, :])
                        nc.sync.dma_start(out=Kg2[c][:, :], in_=Kg[c][64:128, :])
                else:
                    Qg2 = Kg2 = None
                Vg = [gpool.tile([L, D], bf16, tag=f"vg{b}", name=f"vg{b}",
                                 bufs=2) for b in range(GB)]
                for b in range(GB):
                    sgv = 2 * g + b // 4
                    bl = slice((b % 4) * L, (b % 4 + 1) * L)
                    psf = psum.tile([128, 512], f32, tag="mm", name="mm", bufs=3)
                    ps = psf[0:L, :]
                    for ci in range(4):
                        nc.tensor.matmul(ps[:, :], hA[ci][sgv][:, bl],
                                         WV[l][ci],
                                         start=(ci == 0),
                                         stop=(ci == 3 and zero_bias))
                    if not zero_bias:
                        nc.tensor.matmul(ps[:, :], ones1L[:, :],
                                         bias_row(l, 2), start=False, stop=True)
                    cp3(b, Vg[b][:, :], ps[:, :])
                return (Qg, Qg2), (Kg, Kg2), Vg

            def attn_batch(QgT, KgT, Vg, sgh, bj, o2):
                """S^T form: psS*[m, (p, l)] = K^T Q + mask^T per head parity.
                Softmax denominators via ones64 col-tiled matmuls; the
                normalization is fused into the PSUM->SBUF o2 evacuation."""
                Qg, Qg2 = QgT
                Kg, Kg2 = KgT
                b = sgh * SGB + bj
                bc = slice(b * L, (b + 1) * L)
                psSe = psum.tile([L, 512], f32, tag="Se", name="Se", bufs=1)
                psSo = psum.tile([L, 512], f32, tag="So", name="So", bufs=1)
                nc.tensor.matmul(psSe[:, 0:4 * L], idt[0:L, 0:L],
                                 mkb[:, :], start=True, stop=False)
                nc.tensor.matmul(psSo[:, 0:4 * L], idt[0:L, 0:L],
                                 mkb[:, :], start=True, stop=False)
                for co in range(4):
                    cb = co * L
                    nc.tensor.matmul(psSe[:, cb:cb + L], Kg[co][0:64, bc],
                                     Qg[co][0:64, bc], start=False,
                                     stop=(co == 3))
                    if KODD:
                        nc.tensor.matmul(psSo[:, cb:cb + L],
                                         Kg[co][64:128, bc],
                                         Qg[co][64:128, bc], start=False,
                                         stop=(co == 3), tile_position=(64, 0))
                    else:
                        nc.tensor.matmul(psSo[:, cb:cb + L],
                                         Kg2[co][0:64, bc],
                                         Qg2[co][0:64, bc], start=False,
                                         stop=(co == 3))
                e = spool.tile([L, 8 * L], bf16, tag="e", name="e")
                nc.scalar.activation(e[:, 0:4 * L], psSe[:, 0:4 * L], AF.Exp)
                nc.scalar.activation(e[:, 4 * L:8 * L], psSo[:, 0:4 * L],
                                     AF.Exp)
                if KATT < 2:
                    return
                psD = psum.tile([128, 512], f32, tag="D", name="D", bufs=1)
                nc.tensor.matmul(psD[0:64, 0:4 * L], ones64[0:L, :],
                                 e[:, 0:4 * L], start=True, stop=True)
                nc.tensor.matmul(psD[64:128, 0:4 * L], ones64[0:L, :],
                                 e[:, 4 * L:8 * L], start=True, stop=True,
                                 tile_position=(0, 64))
                # r = exp(-ln(d)) on ScalarE: same act-table set as exp
                # (natural_log_exp_and_others); DVE reciprocal is 8 cyc/elem.
                lnD = spool.tile([128, 4 * L], f32, tag="lnD", name="lnD",
                                 bufs=2)
                nc.scalar.activation(lnD[:, :], psD[:, 0:4 * L], AF.Ln)
                rB = spool.tile([128, 4 * L], f32, tag="rB", name="rBatt",
                                bufs=2)
                nc.scalar.activation(rB[:, :], lnD[:, :], AF.Exp, scale=-1.0)
                if KATT < 4:
                    return
                psOb = psum.tile([128, 512], f32, tag="Ob", name="Ob", bufs=1)
                for p in range(4):
                    nc.tensor.matmul(
                        psOb[0:64, p * L:(p + 1) * L],
                        Vg[b][:, (2 * p) * DH:(2 * p + 1) * DH],
                        e[:, p * L:(p + 1) * L],
                        start=True, stop=True)
                    nc.tensor.matmul(
                        psOb[64:128, p * L:(p + 1) * L],
                        Vg[b][:, (2 * p + 1) * DH:(2 * p + 2) * DH],
                        e[:, 4 * L + p * L:4 * L + (p + 1) * L],
                        start=True, stop=True, tile_position=(0, 64))
                nc.vector.tensor_mul(
                    o2[:, :].rearrange("q (p w) -> q p w", p=4)
                    [:, :, bj * L:(bj + 1) * L],
                    psOb[:, 0:4 * L].rearrange("q (p m) -> q p m", p=4),
                    rB[:, :].rearrange("q (p m) -> q p m", p=4))

            def do_wo(l, sg, o2):
                psZ = []
                for co in range(4):
                    ps = psum.tile([128, 512], f32, tag="mm", name="mm", bufs=3)
                    for p in range(4):
                        nc.tensor.matmul(ps[:, 0:SGW],
                                         WO[l][p][:, co * 128:(co + 1) * 128],
                                         o2[:, p * SGW:(p + 1) * SGW],
                                         start=(p == 0), stop=False)
                    if not zero_bias:
                        nc.tensor.matmul(ps[:, 0:SGW],
                                         BIAS[l][3:4, co * 128:(co + 1) * 128],
                                         onesRow[:, 0:SGW], start=False,
                                         stop=False)
                    nc.tensor.matmul(ps[:, 0:SGW], idt[:, :],
                                     hA[co][sg][:, :], start=False, stop=True)
                    psZ.append(ps)
                return psZ

            def do_ffn(l, sg):
                cols = slice(sg * SGW, (sg + 1) * SGW)
                Yg = []
                for co in range(4):
                    # borrow the attention-phase PSUM banks (idle in pass B)
                    ps = psum.tile([128, 512], f32,
                                   tag=("Se", "So", "D", "Ob")[co],
                                   name="ffn1", bufs=1)
                    wts = [C1[l][ci][:, co * 128:(co + 1) * 128]
                           for ci in range(4)]
                    bias_ap = (None if zero_bias else
                               BIAS[l][4:5, co * 128:(co + 1) * 128])
                    accum_mm(ps[:, 0:SGW], wts,
                             lambda ci: hB[ci][sg][:, :], bias_ap)
                    yt = zpool.tile([128, SGW], bf16, tag=f"y{co}",
                                    name=f"y{co}")
                    nc.scalar.activation(yt[:, :], ps[:, 0:SGW], AF.Gelu)
                    Yg.append(yt)
                psZ2 = []
                for co in range(4):
                    ps = psum.tile([128, 512], f32, tag="mm", name="mm", bufs=3)
                    for ci in range(4):
                        nc.tensor.matmul(ps[:, 0:SGW],
                                         C2[l][ci][:, co * 128:(co + 1) * 128],
                                         Yg[ci][:, :], start=(ci == 0),
                                         stop=False)
                    if not zero_bias:
                        nc.tensor.matmul(ps[:, 0:SGW],
                                         BIAS[l][5:6, co * 128:(co + 1) * 128],
                                         onesRow[:, 0:SGW], start=False,
                                         stop=False)
                    nc.tensor.matmul(ps[:, 0:SGW], idt[:, :],
                                     hB[co][sg][:, :], start=False, stop=True)
                    psZ2.append(ps)
                return psZ2

            # ---- layers (subgroup-local pipeline) -----------------------
            for l in range(E if KPHASE >= 2 else 0):
                ln1, ln2 = 2 * l, 2 * l + 1
                load_weights(l)
                # pass A: attention + LN1 for all subgroups (exp act-table)
                for g in range(NG):
                    Qg, Kg, Vg = do_qkv(l, g)
                    if KPHASE < 3:
                        continue
                    for sgh in range(2):
                        sg = 2 * g + sgh
                        o2 = gpool.tile([128, 4 * SGW], bf16, tag=f"o2{sgh}",
                                        name=f"o2{sgh}", bufs=2)
                        for bj in range(SGB):
                            attn_batch(Qg, Kg, Vg, sgh, bj, o2)
                        if KPHASE < 4:
                            continue
                        psZ = do_wo(l, sg, o2)
                        if KPHASE < 5:
                            continue
                        zp = [zppool.tile([128, SGW], bf16,
                                          tag=f"zpA_{sg % 4}_{c}",
                                          name=f"zpA{l}_{sg}_{c}")
                              for c in range(4)]
                        ln_front(ln1, sg, psZ,
                                 [hA[c][sg][:, :] for c in range(4)], zp)
                        if KLN >= 2:
                            ln_rstd(ln1, sg)
                        if KLN >= 3:
                            ln_apply(sg, ln1, zp, hB,
                                     None if trivial_affine else AFFT[l][0])
                if KPHASE < 6:
                    continue
                # pass B: FFN + LN2 for all subgroups (gelu act-table)
                for sg in range(NSG):
                    psZ2 = do_ffn(l, sg)
                    zp2 = [zppool.tile([128, SGW], bf16,
                                       tag=f"zpA_{sg % 4}_{c}",
                                       name=f"zpB{l}_{sg}_{c}")
                           for c in range(4)]
                    ln_front(ln2, sg, psZ2,
                             [hB[c][sg][:, :] for c in range(4)], zp2)
                    ln_rstd(ln2, sg)
                    ln_apply(sg, ln2, zp2, hA,
                             None if trivial_affine else AFFT[l][1])
            # end layers

            # ---- final LN + projection ---------------------------------
            if KPHASE >= 2 and KFINAL:
                lnf = 2 * E
                for sg in range(NSG):
                    cols = slice(sg * SGW, (sg + 1) * SGW)
                    zpf = [zppool.tile([128, SGW], bf16,
                                       tag=f"zpF_{sg % 2}_{c}",
                                       name=f"zpf_{sg}_{c}")
                           for c in range(4)]
                    ln_front(lnf, sg, None,
                             [hA[c][sg][:, :] for c in range(4)], zpf)
                    ln_rstd(lnf, sg)
                    rw = lpool.tile([1, SGW], fp16, tag="rw", name="rw",
                                    bufs=4)
                    nc.sync.dma_start(
                        out=rw[0:1, :],
                        in_=rowd[lnf, sg * SGW:(sg + 1) * SGW].unsqueeze(0))
                    rB = lpool.tile([128, SGW], fp16, tag="rB", name="rB",
                                    bufs=4)
                    nc.gpsimd.partition_broadcast(rB[:, :], rw[0:1, :])
                    psf = psum.tile([128, 512], f32,
                                    tag=("Se", "So", "D", "Ob")[sg % 4],
                                    name="proj", bufs=1)
                    ps = psf[0:COUT, :]
                    for ci in range(4):
                        nc.tensor.matmul(ps[:, 0:SGW], pjt[ci][:, :],
                                         zpf[ci][:, :],
                                         start=(ci == 0),
                                         stop=(ci == 3 and zero_bias))
                    if not zero_bias:
                        nc.tensor.matmul(ps[:, 0:SGW], pjb[:, :],
                                         onesRow[:, 0:SGW],
                                         start=False, stop=True)
                    osb = opool.tile([COUT, SGW], f32, tag="osb", name="osb")
                    nc.vector.tensor_mul(osb[:, :], ps[0:COUT, 0:SGW],
                                         rB[0:COUT, :])
                    nc.sync.dma_start(out=out_d[:, cols], in_=osb[:, :])
            if KPHASE < 2 or not KFINAL:
                # debug: dump embed output
                for sg in range(NSG):
                    cols = slice(sg * SGW, (sg + 1) * SGW)
                    osb = opool.tile([COUT, SGW], f32, tag="osb", name="osb")
                    nc.vector.tensor_copy(osb[:, :], hA[0][sg][0:COUT, :])
                    nc.sync.dma_start(out=out_d[:, cols], in_=osb[:, :])

    nc.compile()
    return nc


# ---------------------------------------------------------------------------
# host side
# ---------------------------------------------------------------------------

def _pos_encoding():
    pos = np.arange(L)[:, None].astype(np.float32)
    div = np.exp(np.arange(0, D, 2).astype(np.float32) * (-np.log(10000.0) / D))
    pe = np.zeros((L, D), dtype=np.float32)
    pe[:, 0::2] = np.sin(pos * div)
    pe[:, 1::2] = np.cos(pos * div)
    return pe


def _chunk4(mT):
    """[D, N] -> [4, 128, N]"""
    return np.ascontiguousarray(mT.reshape(4, 128, -1))


_NC = None
_NC_FLAGS = None


def _get_nc(trivial_affine=True, zero_bias=True):
    global _NC, _NC_FLAGS
    if _NC is None or _NC_FLAGS != (trivial_affine, zero_bias):
        _NC = build_nc(trivial_affine, zero_bias)
        _NC_FLAGS = (trivial_affine, zero_bias)
    return _NC


def is_trivial_affine(inputs):
    i = {k: np.asarray(v) for k, v in inputs.items()}
    return (np.all(i["ln1s"] == 1.0) and np.all(i["ln1b"] == 0.0)
            and np.all(i["ln2s"] == 1.0) and np.all(i["ln2b"] == 0.0))


def is_zero_bias(inputs):
    i = {k: np.asarray(v) for k, v in inputs.items()}
    return all(bool(np.all(i[k] == 0.0))
               for k in ("bq", "bk", "bv", "bo", "c1b", "c2b", "proj_b",
                         "lnfb"))


def prepare_maps(inputs):
    inp = {k: np.asarray(v) for k, v in inputs.items()}
    x = inp["x"].astype(np.float32)
    emb_w = inp["emb_w"].astype(np.float32)
    mask = inp["mask"].astype(np.float32)

    scale = 1.0 / np.sqrt(DH)

    wqt = np.stack([_chunk4(inp["Wq"][l].T * scale) for l in range(E)]).astype(BF16)
    wkt = np.stack([_chunk4(inp["Wk"][l].T) for l in range(E)]).astype(BF16)
    wvt = np.stack([_chunk4(inp["Wv"][l].T) for l in range(E)]).astype(BF16)
    wot = np.stack([_chunk4(inp["Wo"][l].T) for l in range(E)]).astype(BF16)
    c1wt = np.stack([_chunk4(inp["c1w"][l].T) for l in range(E)]).astype(BF16)
    c2wt = np.stack([_chunk4(inp["c2w"][l].T) for l in range(E)]).astype(BF16)

    biasd = np.zeros((E, 8, D), np.float32)
    for l in range(E):
        biasd[l, 0] = inp["bq"][l] * scale
        biasd[l, 1] = inp["bk"][l]
        biasd[l, 2] = inp["bv"][l]
        biasd[l, 3] = inp["bo"][l]
        biasd[l, 4] = inp["c1b"][l]
        biasd[l, 5] = inp["c2b"][l]
    affd = np.zeros((E, 2, 2, 4, 128), np.float32)
    for l in range(E):
        affd[l, 0, 0] = inp["ln1s"][l].reshape(4, 128)
        affd[l, 0, 1] = inp["ln1b"][l].reshape(4, 128)
        affd[l, 1, 0] = inp["ln2s"][l].reshape(4, 128)
        affd[l, 1, 1] = inp["ln2b"][l].reshape(4, 128)

    projw_eff = inp["proj_w"] * inp["lnfs"][None, :]
    projb_eff = inp["proj_b"] + inp["lnfb"] @ inp["proj_w"].T
    projt = np.ascontiguousarray(projw_eff.T.reshape(4, 128, COUT)).astype(BF16)

    pet = np.ascontiguousarray(
        np.tile(_pos_encoding().T.reshape(4, 128, L), (1, 1, SGB))).astype(BF16)
    wcat = np.concatenate([emb_w[:, :, 0].T, emb_w[:, :, 1].T,
                           emb_w[:, :, 2].T], axis=0)
    ident = np.eye(128, dtype=np.float32).astype(BF16)

    onesdd = np.concatenate([np.full((128, 128), 1.0 / D, np.float32),
                             np.ones((128, 64), np.float32),
                             np.ones((128, 32), np.float32)], axis=1)
    shared = dict(
        onesdd=onesdd.astype(BF16),
        wcat=wcat.astype(BF16), petd=pet, wqt=wqt, wkt=wkt, wvt=wvt, wot=wot,
        c1wt=c1wt, c2wt=c2wt, m01d=mask.astype(BF16),
        maskbd=np.tile(-30.0 * (1.0 - mask).T, (1, 4)).astype(BF16),
        identd=ident,
        projt=projt, biasd=biasd.astype(BF16),
        projbd=projb_eff.reshape(1, COUT).astype(BF16), affd=affd,
    )

    in_maps = []
    for ci in range(NC_CORES):
        xs = x[ci * BL:(ci + 1) * BL]                      # [32, 100, 38]
        xp = np.concatenate([xs[:, -1:], xs, xs[:, :1]], axis=1)  # [32,102,38]
        feats = [xp[:, w:w + L, :] for w in range(3)]      # each [32,100,38]
        xaug = np.concatenate(feats, axis=2)               # [32,100,114]
        xaugT = np.ascontiguousarray(
            xaug.reshape(T, KC).T).astype(BF16)            # [114, 3200]
        m = dict(shared)
        m["xaugT"] = xaugT
        in_maps.append(m)
    return in_maps


def run(inputs, **kw):
    nc = _get_nc(is_trivial_affine(inputs), is_zero_bias(inputs))
    in_maps = prepare_maps(inputs)
    res = run_bass_kernel_spmd(nc, in_maps, core_ids=list(range(NC_CORES)), **kw)
    outs = []
    for ci in range(NC_CORES):
        o = np.asarray(res.results[ci]["out"], np.float32)  # [38, 3200]
        outs.append(o.T.reshape(BL, L, COUT))
    full = np.concatenate(outs, axis=0)
    return full, res


def kernel(**inputs):
    full, _ = run(inputs)
    return full.astype(np.float32)


def bench(inputs, iters=6):
    """Steady-state wall timing of the sharded jitted executable."""
    import time
    import jax
    from jax.sharding import Mesh, PartitionSpec
    from jax.experimental.shard_map import shard_map
    from concourse import mybir
    from concourse.bass2jax import _bass_exec_p, install_neuronx_cc_hook, partition_id_tensor

    nc = _get_nc(is_trivial_affine(inputs), is_zero_bias(inputs))
    in_maps = prepare_maps(inputs)
    install_neuronx_cc_hook()
    partition_name = nc.partition_id_tensor.name if nc.partition_id_tensor else None
    in_names, out_names, out_avals, zero_outs = [], [], [], []
    for alloc in nc.m.functions[0].allocations:
        if not isinstance(alloc, mybir.MemoryLocationSet):
            continue
        name = alloc.memorylocations[0].name
        if alloc.kind == "ExternalInput":
            if name != partition_name:
                in_names.append(name)
        elif alloc.kind == "ExternalOutput":
            out_names.append(name)
            shape = tuple(alloc.tensor_shape)
            dtype = mybir.dt.np(alloc.dtype)
            out_avals.append(jax.core.ShapedArray(shape, dtype))
            zero_outs.append(np.zeros(shape, dtype))
    n_params = len(in_names)
    n_outs = len(out_avals)
    all_names = list(in_names) + out_names + ([partition_name] if partition_name else [])

    def _body(*args):
        operands = list(args)
        if partition_name is not None:
            operands.append(partition_id_tensor())
        return tuple(_bass_exec_p.bind(
            *operands, out_avals=tuple(out_avals), in_names=tuple(all_names),
            out_names=tuple(out_names), lowering_input_output_aliases=(),
            sim_require_finite=True, sim_require_nnan=True, nc=nc))

    devices = jax.devices()[:NC_CORES]
    mesh = Mesh(np.array(devices), ("core",))
    donate = tuple(range(n_params, n_params + n_outs))
    sharded = jax.jit(
        shard_map(_body, mesh=mesh,
                  in_specs=(PartitionSpec("core"),) * (n_params + n_outs),
                  out_specs=(PartitionSpec("core"),) * n_outs,
                  check_rep=False),
        donate_argnums=donate, keep_unused=True)
    concat_in = [np.concatenate([np.asarray(in_maps[c][n]) for c in range(NC_CORES)], axis=0)
                 for n in in_names]
    dev_in = [jax.device_put(a) for a in concat_in]
    times = []
    out = None
    for it in range(iters):
        zeros = [jax.device_put(np.zeros((NC_CORES * z.shape[0], *z.shape[1:]), z.dtype))
                 for z in zero_outs]
        jax.block_until_ready(zeros)
        t0 = time.perf_counter()
        out = sharded(*dev_in, *zeros)
        jax.block_until_ready(out)
        times.append(time.perf_counter() - t0)
    res = np.asarray(out[0]).reshape(NC_CORES, COUT, T)
    full = np.concatenate([res[c].T.reshape(BL, L, COUT) for c in range(NC_CORES)], axis=0)
    return full, times

